# revision 9
# baseline (speedup 1.0000x reference)
"""Bidirectional LSTM on 8 trn2 NeuronCores.

Sharding: 2 directions x 4-way batch split (B_local=8 per core). Every core
runs the IDENTICAL forward-scan program; backward cores receive
time-reversed x and their outputs are re-reversed on the host. The scan is
fully core-local.

Per-core plan (B=8, T=512, I=256, H=512, G=4H=2048):
  1. Host pre-transposes/casts weights and x to fp16 (lhsT / moving-operand
     layouts). x is shipped t-major ([I, T*B]) so xp's step-t columns are
     contiguous.
  2. Precompute xp = x @ W_ih.T + b for all T into an SBUF-resident fp16
     buffer (gates.T layout, t-major).
  3. 512-step scan. Per step the gate pre-activations live in PSUM:
     xp is injected into each gate's PSUM bank by an identity matmul
     (start=True) issued during the PREVIOUS step's PE tail-idle, then the
     16 W_hh matmuls per gate accumulate on top (start=False). ACT reads
     PSUM directly (no DVE add on the critical path). Bank packing:
     ps_g [128,32] alone (read early), ps_if [128,64] (i|f read together
     after f), ps_o [128,32] alone (read last) -- avoids fatal PE-write/
     ACT-read same-bank collisions. DVE chain per step is 3 ops:
     [i*g|f*c] as one 64-col mul (tanh_g and c_prev packed adjacent in one
     'gc' tile), c_new add, h mul. h is written fp16 directly into the
     windowed output tile and next step's matmuls stream it from there.
  4. Output windows DMA'd to DRAM fp16, unscrambled and upcast on host.

The compiled PJRT executable is cached at module level: repeat kernel()
calls only transfer fresh inputs and execute.
"""

import numpy as np

B_FULL, T, I, H = 32, 512, 256, 512
G = 4 * H
N_CORES = 8
B = B_FULL // 4          # per-core batch
KH = H // 128            # 4 k-chunks for W_hh
KI = I // 128            # 2 k-chunks for W_ih
M = G // 128             # 16 m-chunks (4 per gate)
WIN = 16                 # scan steps per output DMA window
T_SCAN = T

_BUILT = {}


def _install_tile_patch():
    """This container's walrus accepts only ONE sync-wait per CTRL-class
    instruction (Drain/NoOp). Tile's kernel-tail drain aggregates one wait
    per semaphore lane onto a single Drain -> split them one per drain."""
    import bass_rust
    import concourse.tile as tile

    if getattr(tile.TileContext, "_drain_split_patched", False):
        return

    def _patched_dab(self, tick_clock, wait_clock):
        from concourse.tile import ScopedClock

        nc = self.nc
        drain_inst = nc.sync.drain()
        wait_clock.add_sem_waits(
            drain_inst.ins, ScopedClock({None: tick_clock.global_clock})
        )
        si = drain_inst.ins.sync_info
        waits = list(si.on_wait) if si is not None else []
        if len(waits) > 1:
            si.on_wait = waits[:1]
            for w in waits[1:]:
                d2 = nc.sync.drain()
                si2 = d2.ins.sync_info
                if si2 is None:
                    d2.ins.sync_info = bass_rust.SyncInfo(on_wait=[w], on_update=[])
                else:
                    si2.on_wait = list(si2.on_wait) + [w]
        nc.all_engine_barrier()
        assert self.sems is not None
        popped = nc._tile_sem_poison_stack.pop()
        assert popped is self._sem_poison
        nc.clear_and_free_semaphores(list(self.sems.allocated().values()))
        nc.all_engine_barrier()

    tile.TileContext._drain_and_barrier = _patched_dab
    tile.TileContext._drain_split_patched = True

    # This walrus build accepts at most ONE sync-wait per instruction (any
    # opcode). Split every multi-wait instruction at BIR-JSON level into
    # single-wait NoOps followed by the real instruction with one wait.
    import json
    import concourse.bass as bass

    if getattr(bass.Bass, "_json_wait_split_patched", False):
        return
    _orig_tjb = bass.Bass.to_json_bytes

    def _split_json(self):
        raw = _orig_tjb(self)
        m = json.loads(raw)
        ctr = 0
        changed = False
        for fn in m.get("functions", []):
            for bb in fn.get("blocks", []):
                out = []
                for inst in bb.get("instructions", []):
                    si = inst.get("sync_info")
                    waits = (si or {}).get("on_wait") or []
                    if len(waits) > 1:
                        changed = True
                        for w in waits[:-1]:
                            ctr += 1
                            nop = {
                                "engine": inst["engine"],
                                "ins": [],
                                "outs": [],
                                "name": f"WSPLIT-{ctr}",
                                "opcode": "NoOp",
                                "sync_info": {"on_update": [], "on_wait": [w]},
                            }
                            if "debug" in inst:
                                nop["debug"] = inst["debug"]
                            out.append(nop)
                        si["on_wait"] = [waits[-1]]
                    out.append(inst)
                bb["instructions"] = out
        if not changed:
            return raw
        return json.dumps(m).encode()

    bass.Bass.to_json_bytes = _split_json
    bass.Bass._json_wait_split_patched = True


def _build(t_scan):
    import concourse.bass as bass
    import concourse.tile as tile
    from concourse import mybir
    from contextlib import ExitStack

    _install_tile_patch()
    f32 = mybir.dt.float32
    f16 = mybir.dt.float16

    nc = bass.Bass()
    # Host pre-transposes/casts: xT [I, T*B] f16 t-major, whhT [H, G] f16,
    # wihT [I, G] f16, b_sb [128, M] f32, eye [128, 128] f16.
    xt_d = nc.dram_tensor("xT", [I, T * B], f16, kind="ExternalInput")
    wiht_d = nc.dram_tensor("wihT", [I, G], f16, kind="ExternalInput")
    whht_d = nc.dram_tensor("whhT", [H, G], f16, kind="ExternalInput")
    bsb_d = nc.dram_tensor("bsb", [128, M], f32, kind="ExternalInput")
    eye_d = nc.dram_tensor("eye", [128, 128], f16, kind="ExternalInput")
    n_win = (t_scan + WIN - 1) // WIN
    out_d = nc.dram_tensor("out_raw", [n_win, 128, WIN * 4 * B], f16,
                           kind="ExternalOutput")

    TB = B * T  # 4096 flattened (t, b) columns, t-major

    with tile.TileContext(nc) as tc, ExitStack() as ctx:
        sig = mybir.ActivationFunctionType.Sigmoid
        tanh = mybir.ActivationFunctionType.Tanh

        wpool = ctx.enter_context(tc.tile_pool(name="w", bufs=1))
        whhT = wpool.tile([128, KH * M * 128], f16)   # tile (k,m) at (k*M+m)*128
        wihT = wpool.tile([128, KI * M * 128], f16)
        xT = wpool.tile([128, KI * TB], f16)          # k-chunk ki at ki*TB
        xp = wpool.tile([128, M * TB], f16)           # chunk m at m*TB, col t*B+b
        b_sb = wpool.tile([128, M], f32)
        eye = wpool.tile([128, 128], f16)
        nc.gpsimd.dma_start(b_sb[:], bsb_d[:])
        nc.gpsimd.dma_start(eye[:], eye_d[:])
        for k in range(KI):
            nc.gpsimd.dma_start(wihT[:, k * G:(k + 1) * G],
                                wiht_d[k * 128:(k + 1) * 128, :])
            nc.gpsimd.dma_start(xT[:, k * TB:(k + 1) * TB],
                                xt_d[k * 128:(k + 1) * 128, :])
        for k in range(KH):
            nc.gpsimd.dma_start(whhT[:, k * G:(k + 1) * G],
                                whht_d[k * 128:(k + 1) * 128, :])

        # ---- phase C: xp = x @ W_ih.T + b, fp16, gates.T layout, t-major --
        NXP = 512
        with tc.tile_pool(name="xppsum", bufs=4, space="PSUM") as xpp:
            for m in range(M):
                for n in range(TB // NXP):
                    ps = xpp.tile([128, NXP], f32, tag="xps")
                    for k in range(KI):
                        nc.tensor.matmul(
                            ps[:],
                            wihT[:, (k * M + m) * 128:(k * M + m + 1) * 128],
                            xT[:, k * TB + n * NXP:k * TB + (n + 1) * NXP],
                            start=(k == 0), stop=(k == KI - 1),
                        )
                    dst = xp[:, m * TB + n * NXP:m * TB + (n + 1) * NXP]
                    if n % 2 == 0:
                        nc.vector.tensor_scalar_add(dst, ps[:], b_sb[:, m:m + 1])
                    else:
                        nc.scalar.add(dst, ps[:], b_sb[:, m:m + 1])

        # ---- phase D: the scan ----
        # xp4[p, m, t, b]; gate m-ranges: i=0:4 f=4:8 g=8:12 o=12:16
        xp4 = xp.rearrange("p (m t b) -> p m t b", m=M, t=T)
        GB = KH * B  # 32 cols per gate, col = 8k + b
        with tc.tile_pool(name="gpsum", bufs=2, space="PSUM") as gp, \
             tc.tile_pool(name="acts", bufs=2) as apool, \
             tc.tile_pool(name="state", bufs=2) as stp, \
             tc.tile_pool(name="outb", bufs=2) as obp:

            # gate order f, i, g, o: f's sigmoid gates the longest chain
            # (fc mul), g's tanh next; o is only needed for the final h mul.
            GATES = ((4, "psF"), (0, "psI"), (8, "psG"), (12, "psO"))

            def id_mms(t):
                """Open step t's PSUM groups with identity-matmul xp loads."""
                only = t == 0  # no W matmuls at t=0 (h_{-1}=0)
                out = []
                for mbase, tag in GATES:
                    # full-bank tile (512 f32 = 2 KiB): forces each slot into
                    # its own PSUM bank so the bank-overlap tracker never
                    # serializes one gate's matmuls against another gate's
                    # ACT read (only cols 0:GB are used).
                    pb = gp.tile([128, 512], f32, tag=tag)
                    nc.tensor.matmul(pb[:, 0:GB], eye[:],
                                     xp4[:, mbase:mbase + 4, t, :],
                                     start=True, stop=only)
                    out.append(pb)
                return out

            pss = id_mms(0)
            c_prev = stp.tile([128, GB], f32, tag="c")
            nc.vector.memset(c_prev[:], 0.0)

            ob = None
            h_tile, h_off = None, 0
            for t in range(t_scan):
                s = t % WIN
                if s == 0:
                    ob = obp.tile([128, WIN * GB], f16, tag="ob")
                if t > 0:
                    # W_hh matmuls accumulate on top of the xp identity load.
                    for (mbase, _), ps in zip(GATES, pss):
                        for mi in range(KH):
                            m = mbase + mi
                            for k in range(KH):
                                nc.tensor.matmul(
                                    ps[:, 8 * mi:8 * mi + 8],
                                    whhT[:, (k * M + m) * 128:
                                         (k * M + m + 1) * 128],
                                    h_tile[:, h_off + 8 * k:h_off + 8 * k + 8],
                                    start=False,
                                    stop=(mi == KH - 1 and k == KH - 1),
                                )
                ps_f, ps_i, ps_g, ps_o = pss
                # ACT chain (FIFO order = readiness order)
                sf = apool.tile([128, GB], f32, tag="sf")
                nc.scalar.activation(sf[:], ps_f[:, 0:GB], sig)
                si = apool.tile([128, GB], f32, tag="si")
                nc.scalar.activation(si[:], ps_i[:, 0:GB], sig)
                tg = apool.tile([128, GB], f32, tag="tg")
                nc.scalar.activation(tg[:], ps_g[:, 0:GB], tanh)
                so = apool.tile([128, GB], f32, tag="so")
                nc.scalar.activation(so[:], ps_o[:, 0:GB], sig)
                # DVE chain
                fc = apool.tile([128, GB], f32, tag="fc")
                nc.vector.tensor_mul(fc[:], sf[:], c_prev[:])
                ig = apool.tile([128, GB], f32, tag="ig")
                nc.vector.tensor_mul(ig[:], si[:], tg[:])
                # prefetch next step's xp into fresh PSUM banks (PE tail idle)
                nxt = id_mms(t + 1) if t + 1 < t_scan else None
                c_new = stp.tile([128, GB], f32, tag="c")
                nc.vector.tensor_add(c_new[:], fc[:], ig[:])
                th = apool.tile([128, GB], f32, tag="th")
                nc.scalar.activation(th[:], c_new[:], tanh)
                nc.vector.tensor_mul(ob[:, GB * s:GB * s + GB], so[:], th[:])
                h_tile, h_off = ob, GB * s
                c_prev = c_new
                if nxt is not None:
                    pss = nxt
                if s == WIN - 1 or t == t_scan - 1:
                    nc.gpsimd.dma_start(out_d[t // WIN], ob[:])

    return nc


def _get_nc(t_scan):
    key = t_scan
    if key not in _BUILT:
        _BUILT[key] = _build(t_scan)
    return _BUILT[key]


_EYE = np.eye(128, dtype=np.float16)


def make_in_maps(x, W_ih_f, W_hh_f, b_f, W_ih_b, W_hh_b, b_b):
    """Per-core input dict list (cores 0-3 fwd batch shards, 4-7 bwd)."""
    x = np.asarray(x, dtype=np.float32)
    params = {}
    for d, (wih, whh, bb) in enumerate(
            [(W_ih_f, W_hh_f, b_f), (W_ih_b, W_hh_b, b_b)]):
        wih = np.asarray(wih, np.float32)
        whh = np.asarray(whh, np.float32)
        bb = np.asarray(bb, np.float32)
        params[d] = (
            np.ascontiguousarray(wih.T).astype(np.float16),     # [I, G]
            np.ascontiguousarray(whh.T).astype(np.float16),     # [H, G]
            np.ascontiguousarray(bb.reshape(M, 128).T),         # [128, M]
        )
    in_maps = []
    for c in range(N_CORES):
        d = c // 4          # 0 = forward, 1 = backward
        bs = (c % 4) * B
        xs = x[bs:bs + B]
        if d == 1:
            xs = xs[:, ::-1]
        # t-major: xT[i, t*B + b] = xs[b, t, i]
        xt = np.ascontiguousarray(
            xs.transpose(2, 1, 0).reshape(I, T * B)).astype(np.float16)
        wiht, whht, bsb = params[d]
        in_maps.append({
            "xT": xt, "wihT": wiht, "whhT": whht, "bsb": bsb, "eye": _EYE,
        })
    return in_maps


_RUNNERS = {}


def _make_runner(t_scan):
    """Compile once, return a callable in_maps -> list[dict] that only
    executes (PJRT executable cached across kernel() calls). Donated output
    buffers are created on-device (jnp.zeros) so they are never shipped
    from the host."""
    import jax
    import jax.numpy as jnp
    import numpy as np
    from jax.sharding import Mesh, PartitionSpec
    from jax.experimental.shard_map import shard_map
    from concourse import bass2jax, mybir
    from concourse.bass2jax import _bass_exec_p, install_neuronx_cc_hook

    install_neuronx_cc_hook()
    nc = _get_nc(t_scan)
    assert nc.dbg_addr is None
    n_cores = N_CORES
    partition_name = (nc.partition_id_tensor.name
                      if nc.partition_id_tensor else None)
    in_names, out_names, out_avals, zero_shapes = [], [], [], []
    for alloc in nc.m.functions[0].allocations:
        if not isinstance(alloc, mybir.MemoryLocationSet):
            continue
        name = alloc.memorylocations[0].name
        if alloc.kind == "ExternalInput":
            if name != partition_name:
                in_names.append(name)
        elif alloc.kind == "ExternalOutput":
            shape = tuple(alloc.tensor_shape)
            npdt = mybir.dt.np(alloc.dtype)
            out_avals.append(jax.core.ShapedArray(shape, npdt))
            out_names.append(name)
            zero_shapes.append((shape, npdt))
    n_params = len(in_names)
    n_outs = len(out_names)
    all_in = in_names + out_names
    if partition_name is not None:
        all_in = all_in + [partition_name]

    def _body(*args):
        operands = list(args)
        if partition_name is not None:
            operands.append(bass2jax.partition_id_tensor())
        outs = _bass_exec_p.bind(
            *operands,
            out_avals=tuple(out_avals),
            in_names=tuple(all_in),
            out_names=tuple(out_names),
            lowering_input_output_aliases=(),
            sim_require_finite=True,
            sim_require_nnan=True,
            nc=nc,
        )
        return tuple(outs)

    devices = jax.devices()[:n_cores]
    mesh = Mesh(np.asarray(devices), ("core",))
    donate = tuple(range(n_params, n_params + n_outs))
    sharded = jax.jit(
        shard_map(_body, mesh=mesh,
                  in_specs=(PartitionSpec("core"),) * (n_params + n_outs),
                  out_specs=(PartitionSpec("core"),) * n_outs,
                  check_rep=False),
        donate_argnums=donate, keep_unused=True,
    )

    def run(in_maps):
        concat_in = [
            np.concatenate([np.asarray(m[name]) for m in in_maps], axis=0)
            for name in in_names
        ]
        concat_zeros = [
            jnp.zeros((n_cores * s[0], *s[1:]), dt) for s, dt in zero_shapes
        ]
        out_arrs = sharded(*concat_in, *concat_zeros)
        return [
            {name: np.asarray(out_arrs[i]).reshape(
                n_cores, *out_avals[i].shape)[c]
             for i, name in enumerate(out_names)}
            for c in range(n_cores)
        ]

    return run


def _run_spmd(t_scan, in_maps):
    if t_scan not in _RUNNERS:
        try:
            _RUNNERS[t_scan] = _make_runner(t_scan)
        except Exception:
            _RUNNERS[t_scan] = None
    runner = _RUNNERS[t_scan]
    if runner is not None:
        return runner(in_maps)
    from concourse.bass_utils import run_bass_kernel_spmd
    res = run_bass_kernel_spmd(_get_nc(t_scan), in_maps, list(range(N_CORES)))
    return res.results


def kernel(x, W_ih_f, W_hh_f, b_f, W_ih_b, W_hh_b, b_b, _t_scan=T_SCAN):
    in_maps = make_in_maps(x, W_ih_f, W_hh_f, b_f, W_ih_b, W_hh_b, b_b)
    results = _run_spmd(_t_scan, in_maps)
    return unscramble(results, _t_scan)


def unscramble(results, _t_scan=T_SCAN):
    n_win = (_t_scan + WIN - 1) // WIN
    t_out = n_win * WIN
    halves = []
    for d in range(2):
        parts = []
        for c4 in range(4):
            raw = np.asarray(results[d * 4 + c4]["out_raw"])
            # raw[w, p, 32s + 8k + b] = h[b, 16w+s, 128k+p]
            h = raw.reshape(n_win, 128, WIN, KH, B)
            h = np.ascontiguousarray(h.transpose(4, 0, 2, 3, 1))
            h = h.reshape(B, t_out, H)[:, :_t_scan]
            parts.append(h)
        hcat = np.concatenate(parts, axis=0)
        if d == 1:
            hcat = hcat[:, ::-1]
        halves.append(hcat)
    return np.concatenate(halves, axis=2).astype(np.float32)


# revision 11
# speedup vs baseline: 1.0013x; 1.0013x over previous
"""Bidirectional LSTM on 8 trn2 NeuronCores.

Sharding: 2 directions x 4-way batch split (B_local=8 per core). Every core
runs the IDENTICAL forward-scan program; backward cores receive
time-reversed x and their outputs are re-reversed on the host. The scan is
fully core-local.

Per-core plan (B=8, T=512, I=256, H=512, G=4H=2048):
  1. Host pre-transposes/casts weights and x to fp16 (lhsT / moving-operand
     layouts). x is shipped t-major ([I, T*B]) so xp's step-t columns are
     contiguous.
  2. Precompute xp = x @ W_ih.T + b for all T into an SBUF-resident fp16
     buffer (gates.T layout, t-major).
  3. 512-step scan. Per step the gate pre-activations live in PSUM:
     xp is injected into each gate's PSUM bank by an identity matmul
     (start=True) issued during the PREVIOUS step's PE tail-idle, then the
     16 W_hh matmuls per gate accumulate on top (start=False). ACT reads
     PSUM directly (no DVE add on the critical path). Bank packing:
     ps_g [128,32] alone (read early), ps_if [128,64] (i|f read together
     after f), ps_o [128,32] alone (read last) -- avoids fatal PE-write/
     ACT-read same-bank collisions. DVE chain per step is 3 ops:
     [i*g|f*c] as one 64-col mul (tanh_g and c_prev packed adjacent in one
     'gc' tile), c_new add, h mul. h is written fp16 directly into the
     windowed output tile and next step's matmuls stream it from there.
  4. Output windows DMA'd to DRAM fp16, unscrambled and upcast on host.

The compiled PJRT executable is cached at module level: repeat kernel()
calls only transfer fresh inputs and execute.
"""

import numpy as np

B_FULL, T, I, H = 32, 512, 256, 512
G = 4 * H
N_CORES = 8
B = B_FULL // 4          # per-core batch
KH = H // 128            # 4 k-chunks for W_hh
KI = I // 128            # 2 k-chunks for W_ih
M = G // 128             # 16 m-chunks (4 per gate)
WIN = 16                 # scan steps per output DMA window
T_SCAN = T

_BUILT = {}


def _install_tile_patch():
    """This container's walrus accepts only ONE sync-wait per CTRL-class
    instruction (Drain/NoOp). Tile's kernel-tail drain aggregates one wait
    per semaphore lane onto a single Drain -> split them one per drain."""
    import bass_rust
    import concourse.tile as tile

    if getattr(tile.TileContext, "_drain_split_patched", False):
        return

    def _patched_dab(self, tick_clock, wait_clock):
        from concourse.tile import ScopedClock

        nc = self.nc
        drain_inst = nc.sync.drain()
        wait_clock.add_sem_waits(
            drain_inst.ins, ScopedClock({None: tick_clock.global_clock})
        )
        si = drain_inst.ins.sync_info
        waits = list(si.on_wait) if si is not None else []
        if len(waits) > 1:
            si.on_wait = waits[:1]
            for w in waits[1:]:
                d2 = nc.sync.drain()
                si2 = d2.ins.sync_info
                if si2 is None:
                    d2.ins.sync_info = bass_rust.SyncInfo(on_wait=[w], on_update=[])
                else:
                    si2.on_wait = list(si2.on_wait) + [w]
        nc.all_engine_barrier()
        assert self.sems is not None
        popped = nc._tile_sem_poison_stack.pop()
        assert popped is self._sem_poison
        nc.clear_and_free_semaphores(list(self.sems.allocated().values()))
        nc.all_engine_barrier()

    tile.TileContext._drain_and_barrier = _patched_dab
    tile.TileContext._drain_split_patched = True

    # This walrus build accepts at most ONE sync-wait per instruction (any
    # opcode). Split every multi-wait instruction at BIR-JSON level into
    # single-wait NoOps followed by the real instruction with one wait.
    import json
    import concourse.bass as bass

    if getattr(bass.Bass, "_json_wait_split_patched", False):
        return
    _orig_tjb = bass.Bass.to_json_bytes

    def _split_json(self):
        raw = _orig_tjb(self)
        m = json.loads(raw)
        ctr = 0
        changed = False
        for fn in m.get("functions", []):
            for bb in fn.get("blocks", []):
                out = []
                for inst in bb.get("instructions", []):
                    si = inst.get("sync_info")
                    waits = (si or {}).get("on_wait") or []
                    if len(waits) > 1:
                        changed = True
                        for w in waits[:-1]:
                            ctr += 1
                            nop = {
                                "engine": inst["engine"],
                                "ins": [],
                                "outs": [],
                                "name": f"WSPLIT-{ctr}",
                                "opcode": "NoOp",
                                "sync_info": {"on_update": [], "on_wait": [w]},
                            }
                            if "debug" in inst:
                                nop["debug"] = inst["debug"]
                            out.append(nop)
                        si["on_wait"] = [waits[-1]]
                    out.append(inst)
                bb["instructions"] = out
        if not changed:
            return raw
        return json.dumps(m).encode()

    bass.Bass.to_json_bytes = _split_json
    bass.Bass._json_wait_split_patched = True


def _build(t_scan):
    import concourse.bass as bass
    import concourse.tile as tile
    from concourse import mybir
    from contextlib import ExitStack

    _install_tile_patch()
    f32 = mybir.dt.float32
    f16 = mybir.dt.float16

    nc = bass.Bass()
    # Host pre-transposes/casts: xT [I, T*B] f16 t-major, whhT [H, G] f16,
    # wihT [I, G] f16, b_sb [128, M] f32, eye [128, 128] f16.
    xt_d = nc.dram_tensor("xT", [I, T * B], f16, kind="ExternalInput")
    wiht_d = nc.dram_tensor("wihT", [I, G], f16, kind="ExternalInput")
    whht_d = nc.dram_tensor("whhT", [H, G], f16, kind="ExternalInput")
    bsb_d = nc.dram_tensor("bsb", [128, M], f32, kind="ExternalInput")
    eye_d = nc.dram_tensor("eye", [128, 128], f16, kind="ExternalInput")
    n_win = (t_scan + WIN - 1) // WIN
    out_d = nc.dram_tensor("out_raw", [n_win, 128, WIN * 4 * B], f16,
                           kind="ExternalOutput")

    TB = B * T  # 4096 flattened (t, b) columns, t-major

    with tile.TileContext(nc) as tc, ExitStack() as ctx:
        sig = mybir.ActivationFunctionType.Sigmoid
        tanh = mybir.ActivationFunctionType.Tanh

        wpool = ctx.enter_context(tc.tile_pool(name="w", bufs=1))
        whhT = wpool.tile([128, KH * M * 128], f16)   # tile (k,m) at (k*M+m)*128
        wihT = wpool.tile([128, KI * M * 128], f16)
        xT = wpool.tile([128, KI * TB], f16)          # k-chunk ki at ki*TB
        xp = wpool.tile([128, M * TB], f16)           # chunk m at m*TB, col t*B+b
        b_sb = wpool.tile([128, M], f32)
        eye = wpool.tile([128, 128], f16)
        nc.gpsimd.dma_start(b_sb[:], bsb_d[:])
        for k in range(KI):
            nc.gpsimd.dma_start(wihT[:, k * G:(k + 1) * G],
                                wiht_d[k * 128:(k + 1) * 128, :])
        # x quartered and k-interleaved: phase C's first blocks unlock after
        # ~1/4 of the x transfer instead of all of it
        QT = TB // 4
        for q in range(4):
            for k in range(KI):
                nc.gpsimd.dma_start(
                    xT[:, k * TB + q * QT:k * TB + (q + 1) * QT],
                    xt_d[k * 128:(k + 1) * 128, q * QT:(q + 1) * QT])
        nc.gpsimd.dma_start(eye[:], eye_d[:])
        for k in range(KH):
            nc.gpsimd.dma_start(whhT[:, k * G:(k + 1) * G],
                                whht_d[k * 128:(k + 1) * 128, :])

        # ---- phase C: xp = x @ W_ih.T + b, fp16, gates.T layout, t-major --
        NXP = 512
        n_blocks = min((t_scan * B + NXP - 1) // NXP, TB // NXP)
        with tc.tile_pool(name="xppsum", bufs=4, space="PSUM") as xpp:
            for n in range(n_blocks):
                for m in range(M):
                    ps = xpp.tile([128, NXP], f32, tag="xps")
                    for k in range(KI):
                        nc.tensor.matmul(
                            ps[:],
                            wihT[:, (k * M + m) * 128:(k * M + m + 1) * 128],
                            xT[:, k * TB + n * NXP:k * TB + (n + 1) * NXP],
                            start=(k == 0), stop=(k == KI - 1),
                        )
                    dst = xp[:, m * TB + n * NXP:m * TB + (n + 1) * NXP]
                    if (n * M + m) % 2 == 0:
                        nc.vector.tensor_scalar_add(dst, ps[:], b_sb[:, m:m + 1])
                    else:
                        nc.scalar.add(dst, ps[:], b_sb[:, m:m + 1])

        # ---- phase D: the scan ----
        # xp4[p, m, t, b]; gate m-ranges: i=0:4 f=4:8 g=8:12 o=12:16
        xp4 = xp.rearrange("p (m t b) -> p m t b", m=M, t=T)
        GB = KH * B  # 32 cols per gate, col = 8k + b
        with tc.tile_pool(name="gpsum", bufs=2, space="PSUM") as gp, \
             tc.tile_pool(name="acts", bufs=2) as apool, \
             tc.tile_pool(name="state", bufs=2) as stp, \
             tc.tile_pool(name="outb", bufs=2) as obp:

            # gate order f, i, g, o: f's sigmoid gates the longest chain
            # (fc mul), g's tanh next; o is only needed for the final h mul.
            GATES = ((4, "psF"), (0, "psI"), (8, "psG"), (12, "psO"))

            def id_mms(t):
                """Open step t's PSUM groups with identity-matmul xp loads."""
                only = t == 0  # no W matmuls at t=0 (h_{-1}=0)
                out = []
                for mbase, tag in GATES:
                    # full-bank tile (512 f32 = 2 KiB): forces each slot into
                    # its own PSUM bank so the bank-overlap tracker never
                    # serializes one gate's matmuls against another gate's
                    # ACT read (only cols 0:GB are used).
                    pb = gp.tile([128, 512], f32, tag=tag)
                    nc.tensor.matmul(pb[:, 0:GB], eye[:],
                                     xp4[:, mbase:mbase + 4, t, :],
                                     start=True, stop=only)
                    out.append(pb)
                return out

            pss = id_mms(0)
            c_prev = stp.tile([128, GB], f32, tag="c")
            nc.vector.memset(c_prev[:], 0.0)

            ob = None
            h_tile, h_off = None, 0
            for t in range(t_scan):
                s = t % WIN
                if s == 0:
                    ob = obp.tile([128, WIN * GB], f16, tag="ob")
                if t > 0:
                    # W_hh matmuls accumulate on top of the xp identity load.
                    for (mbase, _), ps in zip(GATES, pss):
                        for mi in range(KH):
                            m = mbase + mi
                            for k in range(KH):
                                nc.tensor.matmul(
                                    ps[:, 8 * mi:8 * mi + 8],
                                    whhT[:, (k * M + m) * 128:
                                         (k * M + m + 1) * 128],
                                    h_tile[:, h_off + 8 * k:h_off + 8 * k + 8],
                                    start=False,
                                    stop=(mi == KH - 1 and k == KH - 1),
                                )
                ps_f, ps_i, ps_g, ps_o = pss
                # ACT chain (FIFO order = readiness order)
                sf = apool.tile([128, GB], f32, tag="sf")
                nc.scalar.activation(sf[:], ps_f[:, 0:GB], sig)
                si = apool.tile([128, GB], f32, tag="si")
                nc.scalar.activation(si[:], ps_i[:, 0:GB], sig)
                tg = apool.tile([128, GB], f32, tag="tg")
                nc.scalar.activation(tg[:], ps_g[:, 0:GB], tanh)
                so = apool.tile([128, GB], f32, tag="so")
                nc.scalar.activation(so[:], ps_o[:, 0:GB], sig)
                # DVE chain
                fc = apool.tile([128, GB], f32, tag="fc")
                nc.vector.tensor_mul(fc[:], sf[:], c_prev[:])
                ig = apool.tile([128, GB], f32, tag="ig")
                nc.vector.tensor_mul(ig[:], si[:], tg[:])
                # prefetch next step's xp into fresh PSUM banks (PE tail idle)
                nxt = id_mms(t + 1) if t + 1 < t_scan else None
                c_new = stp.tile([128, GB], f32, tag="c")
                nc.vector.tensor_add(c_new[:], fc[:], ig[:])
                th = apool.tile([128, GB], f32, tag="th")
                nc.scalar.activation(th[:], c_new[:], tanh)
                nc.vector.tensor_mul(ob[:, GB * s:GB * s + GB], so[:], th[:])
                h_tile, h_off = ob, GB * s
                c_prev = c_new
                if nxt is not None:
                    pss = nxt
                if s == WIN - 1 or t == t_scan - 1:
                    nc.gpsimd.dma_start(out_d[t // WIN], ob[:])

    return nc


def _get_nc(t_scan):
    key = t_scan
    if key not in _BUILT:
        _BUILT[key] = _build(t_scan)
    return _BUILT[key]


_EYE = np.eye(128, dtype=np.float16)


def make_in_maps(x, W_ih_f, W_hh_f, b_f, W_ih_b, W_hh_b, b_b):
    """Per-core input dict list (cores 0-3 fwd batch shards, 4-7 bwd)."""
    x = np.asarray(x, dtype=np.float32)
    params = {}
    for d, (wih, whh, bb) in enumerate(
            [(W_ih_f, W_hh_f, b_f), (W_ih_b, W_hh_b, b_b)]):
        wih = np.asarray(wih, np.float32)
        whh = np.asarray(whh, np.float32)
        bb = np.asarray(bb, np.float32)
        params[d] = (
            np.ascontiguousarray(wih.T).astype(np.float16),     # [I, G]
            np.ascontiguousarray(whh.T).astype(np.float16),     # [H, G]
            np.ascontiguousarray(bb.reshape(M, 128).T),         # [128, M]
        )
    in_maps = []
    for c in range(N_CORES):
        d = c // 4          # 0 = forward, 1 = backward
        bs = (c % 4) * B
        xs = x[bs:bs + B]
        if d == 1:
            xs = xs[:, ::-1]
        # t-major: xT[i, t*B + b] = xs[b, t, i]
        xt = np.ascontiguousarray(
            xs.transpose(2, 1, 0).reshape(I, T * B)).astype(np.float16)
        wiht, whht, bsb = params[d]
        in_maps.append({
            "xT": xt, "wihT": wiht, "whhT": whht, "bsb": bsb, "eye": _EYE,
        })
    return in_maps


_RUNNERS = {}


def _make_runner(t_scan):
    """Compile once, return a callable in_maps -> list[dict] that only
    executes (PJRT executable cached across kernel() calls). Donated output
    buffers are created on-device (jnp.zeros) so they are never shipped
    from the host."""
    import jax
    import jax.numpy as jnp
    import numpy as np
    from jax.sharding import Mesh, PartitionSpec
    from jax.experimental.shard_map import shard_map
    from concourse import bass2jax, mybir
    from concourse.bass2jax import _bass_exec_p, install_neuronx_cc_hook

    install_neuronx_cc_hook()
    nc = _get_nc(t_scan)
    assert nc.dbg_addr is None
    n_cores = N_CORES
    partition_name = (nc.partition_id_tensor.name
                      if nc.partition_id_tensor else None)
    in_names, out_names, out_avals, zero_shapes = [], [], [], []
    for alloc in nc.m.functions[0].allocations:
        if not isinstance(alloc, mybir.MemoryLocationSet):
            continue
        name = alloc.memorylocations[0].name
        if alloc.kind == "ExternalInput":
            if name != partition_name:
                in_names.append(name)
        elif alloc.kind == "ExternalOutput":
            shape = tuple(alloc.tensor_shape)
            npdt = mybir.dt.np(alloc.dtype)
            out_avals.append(jax.core.ShapedArray(shape, npdt))
            out_names.append(name)
            zero_shapes.append((shape, npdt))
    n_params = len(in_names)
    n_outs = len(out_names)
    all_in = in_names + out_names
    if partition_name is not None:
        all_in = all_in + [partition_name]

    def _body(*args):
        operands = list(args)
        if partition_name is not None:
            operands.append(bass2jax.partition_id_tensor())
        outs = _bass_exec_p.bind(
            *operands,
            out_avals=tuple(out_avals),
            in_names=tuple(all_in),
            out_names=tuple(out_names),
            lowering_input_output_aliases=(),
            sim_require_finite=True,
            sim_require_nnan=True,
            nc=nc,
        )
        return tuple(outs)

    devices = jax.devices()[:n_cores]
    mesh = Mesh(np.asarray(devices), ("core",))
    donate = tuple(range(n_params, n_params + n_outs))
    sharded = jax.jit(
        shard_map(_body, mesh=mesh,
                  in_specs=(PartitionSpec("core"),) * (n_params + n_outs),
                  out_specs=(PartitionSpec("core"),) * n_outs,
                  check_rep=False),
        donate_argnums=donate, keep_unused=True,
    )

    def run(in_maps):
        concat_in = [
            np.concatenate([np.asarray(m[name]) for m in in_maps], axis=0)
            for name in in_names
        ]
        concat_zeros = [
            jnp.zeros((n_cores * s[0], *s[1:]), dt) for s, dt in zero_shapes
        ]
        out_arrs = sharded(*concat_in, *concat_zeros)
        return [
            {name: np.asarray(out_arrs[i]).reshape(
                n_cores, *out_avals[i].shape)[c]
             for i, name in enumerate(out_names)}
            for c in range(n_cores)
        ]

    return run


def _run_spmd(t_scan, in_maps):
    if t_scan not in _RUNNERS:
        try:
            _RUNNERS[t_scan] = _make_runner(t_scan)
        except Exception:
            _RUNNERS[t_scan] = None
    runner = _RUNNERS[t_scan]
    if runner is not None:
        return runner(in_maps)
    from concourse.bass_utils import run_bass_kernel_spmd
    res = run_bass_kernel_spmd(_get_nc(t_scan), in_maps, list(range(N_CORES)))
    return res.results


def kernel(x, W_ih_f, W_hh_f, b_f, W_ih_b, W_hh_b, b_b, _t_scan=T_SCAN):
    in_maps = make_in_maps(x, W_ih_f, W_hh_f, b_f, W_ih_b, W_hh_b, b_b)
    results = _run_spmd(_t_scan, in_maps)
    return unscramble(results, _t_scan)


def unscramble(results, _t_scan=T_SCAN):
    n_win = (_t_scan + WIN - 1) // WIN
    t_out = n_win * WIN
    halves = []
    for d in range(2):
        parts = []
        for c4 in range(4):
            raw = np.asarray(results[d * 4 + c4]["out_raw"])
            # raw[w, p, 32s + 8k + b] = h[b, 16w+s, 128k+p]
            h = raw.reshape(n_win, 128, WIN, KH, B)
            h = np.ascontiguousarray(h.transpose(4, 0, 2, 3, 1))
            h = h.reshape(B, t_out, H)[:, :_t_scan]
            parts.append(h)
        hcat = np.concatenate(parts, axis=0)
        if d == 1:
            hcat = hcat[:, ::-1]
        halves.append(hcat)
    return np.concatenate(halves, axis=2).astype(np.float32)
